# revision 4
# baseline (speedup 1.0000x reference)
"""AVWDCRNN (adaptive-graph conv GRU, 2 layers) on 8 Trainium2 NeuronCores.

Sharding: data-parallel over batch B=32 -> 4 per core (8 cores).
node_embeddings, weight pools and the [K,N,N] Chebyshev supports are
replicated on every core; each core runs the full T=24 recurrence for its
batch shard; outputs are gathered (concatenated) on the host.  This matches
the problem's sharding hint; no cross-core communication is needed.

The T=24 GRU recurrence is rolled on the host (one jitted step function per
layer-shape) because the Neuron compiler's instruction budget cannot hold the
fully-unrolled scan: the node-adaptive einsum bnkc,nkco->bno lowers to 1024
batched small matmuls per call and 24x that exceeds the 150k-instruction cap.
Host-side rolling keeps every compile unit small; async pmap dispatch keeps
all 8 cores busy back-to-back.

Self-contained: shapes hardcoded (B=32,T=24,N=1024,DIN=4,DOUT=64,ED=16,HD=32).
"""

import functools

import jax
import jax.numpy as jnp
import numpy as np

CHEB_K = 3
NUM_LAYERS = 2
N_CORES = 8

B, T, N, DIN, DOUT, ED, HD = 32, 24, 1024, 4, 64, 16, 32
NB = B // N_CORES                                               # 4 per core


def _cheb_supports(E):
    A = jax.nn.softmax(jax.nn.relu(E @ E.T), axis=1)            # [N,N]
    n = A.shape[0]
    sup = [jnp.eye(n, dtype=A.dtype), A]
    for _ in range(2, CHEB_K):
        sup.append(2.0 * (A @ sup[-1]) - sup[-2])
    return jnp.stack(sup[:CHEB_K], 0)                            # [K,N,N]


@functools.partial(jax.pmap, in_axes=(0, None, None, None, None, None))
def _prep(E, gp0, up0, gp1, up1, gb_ub):
    """Per-device precompute: supports + node-adaptive weights (replicated)."""
    gb0, ub0, gb1, ub1 = gb_ub
    supports = _cheb_supports(E)
    Wg0 = jnp.einsum('nd,dkio->nkio', E, gp0)
    Wu0 = jnp.einsum('nd,dkio->nkio', E, up0)
    Wg1 = jnp.einsum('nd,dkio->nkio', E, gp1)
    Wu1 = jnp.einsum('nd,dkio->nkio', E, up1)
    return (supports, Wg0, E @ gb0, Wu0, E @ ub0,
            Wg1, E @ gb1, Wu1, E @ ub1)


@functools.partial(jax.pmap, in_axes=(0, 0, None, None, None, None))
def _masks_xm0(x, h, mw0, mb0, mw1, mb1):
    """mask_l [T,nb,N,1] for both layers and xm for layer 0 [T,nb,N,DIN]."""
    h_seq = jnp.moveaxis(h, 2, 0)                                # [T,nb,N,HD]
    m0 = jax.nn.sigmoid(h_seq @ mw0 + mb0)                       # [T,nb,N,1]
    m1 = jax.nn.sigmoid(h_seq @ mw1 + mb1)
    x_seq = jnp.swapaxes(x, 0, 1)                                # [T,nb,N,DIN]
    return m0, m1, x_seq * m0


def _avwgcn(x, supports, W, b):
    # x [nb,N,C], supports [K,N,N], W [N,K,C,O], b [N,O]
    xg = jnp.einsum('knm,bmc->bnkc', supports, x)
    return jnp.einsum('bnkc,nkco->bno', xg, W) + b


@functools.partial(jax.pmap, in_axes=(0, 0, 0, 0, 0, 0, 0))
def _step(state, xm_t, supports, Wg, bg, Wu, bu):
    zr = jax.nn.sigmoid(
        _avwgcn(jnp.concatenate([xm_t, state], -1), supports, Wg, bg))
    z, r = jnp.split(zr, 2, axis=-1)
    hc = jnp.tanh(
        _avwgcn(jnp.concatenate([xm_t, r * state], -1), supports, Wu, bu))
    return z * state + (1.0 - z) * hc


@functools.partial(jax.pmap, in_axes=(0, 0))
def _apply_mask(states, mask):
    return states * mask


def kernel(x, init_state, node_embeddings, h,
           gate_pool_0, gate_bias_0, update_pool_0, update_bias_0,
           mask_w_0, mask_b_0,
           gate_pool_1, gate_bias_1, update_pool_1, update_bias_1,
           mask_w_1, mask_b_1):
    xs = np.ascontiguousarray(x.reshape(N_CORES, NB, T, N, DIN))
    ss = np.ascontiguousarray(
        init_state.reshape(NUM_LAYERS, N_CORES, NB, N, DOUT).swapaxes(0, 1))
    hs = np.ascontiguousarray(h.reshape(N_CORES, NB, N, T, HD))

    E8 = np.ascontiguousarray(
        np.broadcast_to(node_embeddings, (N_CORES,) + node_embeddings.shape))
    (supports, Wg0, bg0, Wu0, bu0, Wg1, bg1, Wu1, bu1) = _prep(
        E8, gate_pool_0, update_pool_0,
        gate_pool_1, update_pool_1,
        (gate_bias_0, update_bias_0, gate_bias_1, update_bias_1))
    m0, m1, xm0 = _masks_xm0(xs, hs, mask_w_0, mask_b_0, mask_w_1, mask_b_1)

    # layer 0
    state = ss[:, 0]
    states0 = []
    for t in range(T):
        state = _step(state, xm0[:, t], supports, Wg0, bg0, Wu0, bu0)
        states0.append(state)
    final0 = state
    states0 = jnp.stack(states0, axis=1)                 # [8,T,nb,N,O]

    # layer 1: input = masked layer-0 states
    xm1 = _apply_mask(states0, m1)                       # [8,T,nb,N,O]
    state = ss[:, 1]
    states1 = []
    for t in range(T):
        state = _step(state, xm1[:, t], supports, Wg1, bg1, Wu1, bu1)
        states1.append(state)
    final1 = state
    states1 = jnp.stack(states1, axis=1)                 # [8,T,nb,N,O]

    out = np.asarray(states1)                            # [8,T,nb,N,O]
    out_full = out.transpose(0, 2, 1, 3, 4).reshape(B, T, N, DOUT)
    finals_full = np.stack(
        [np.asarray(final0).reshape(B, N, DOUT),
         np.asarray(final1).reshape(B, N, DOUT)], axis=0)
    masks_full = np.concatenate(
        [np.asarray(m0), np.asarray(m1)], axis=1)        # [8,2T,nb,N,1]
    masks_full = masks_full.transpose(1, 0, 2, 3, 4).reshape(
        NUM_LAYERS * T, B, N, 1)
    return (out_full.astype(np.float32),
            finals_full.astype(np.float32),
            masks_full.astype(np.float32))
